# revision 12
# baseline (speedup 1.0000x reference)
"""NetVLAD Trainium2 Bass kernel.

Full computation (per sample n, C=128 channels, S=4096 spatial, K=64 clusters):
  xn = x / ||x||_channels                       (per spatial position)
  logits[s,k] = 2a * xn_s . cent_k - a*||cent_k||
  a = softmax_k(logits)
  vlad[k,c] = sum_s a[s,k] * (xn[s,c] - cent[k,c])
  vlad <- intra-normalize rows, flatten, l2-normalize

Sharding: data-parallel over batch N=32 across 8 cores (4 samples/core),
centroids replicated. No collectives; host concatenates shard outputs.

Per-core dataflow (per sample, 32 s-tiles of 128 grouped by 4):
  - PE transposes x tiles [c,s]->[s,c] (fp32, exact)
  - ACT square+accum on transposed PSUM -> ssq; r2 = 2a*rsqrt(ssq)
  - PE GEMM1: d[s,k] = x_tile^T @ centT (fp32, exact)
  - softmax trick: a_k ~ exp(r2*(d_k - dmax_s)) * B_k,
      B_k = exp(bias_k - max_j bias_j) precomputed on device.
    dmax rowmax on DVE; exp on ACT (fused scale/bias APs) or DVE+big-exp.
  - t = e*B (GPSIMD); ssum; xn2 = x_sc * (r*inv) with an extra column = inv
  - PE GEMM2 (bf16): V[k, 0:128] += t^T @ xn2 ; V[k,128] = asum
  - epilogue: vlad = V - asum*cent, intra-norm, global norm, DMA out.
"""

import sys

import numpy as np

sys.path.insert(0, "/opt/trn_rl_repo")

import concourse.bacc as bacc  # noqa: E402
import concourse.bass as bass  # noqa: E402
import concourse.tile as tile  # noqa: E402
from concourse import masks, mybir  # noqa: E402
from concourse.bass_utils import run_bass_kernel_spmd  # noqa: E402

F32 = mybir.dt.float32
F16 = mybir.dt.float16
BF16 = mybir.dt.bfloat16
AF = mybir.ActivationFunctionType
ALU = mybir.AluOpType

ALPHA = 100.0
N_CORES = 8
NS = 4  # samples per core
C = 128
S = 4096
K = 64
ST = 128  # s-tile (PSUM partition limit)
GT = 4  # tiles per group
NG = S // (ST * GT)  # 8 groups per sample

# tunables: which of the 4 tiles in a group use the ACT-fused exp
# (others: DVE tensor_scalar u + one big ACT exp)
EXP_FUSED = (2, 3)
EXP_DVE = (0, 1)


def build_nc(compile=True):
    nc = bacc.Bacc("TRN2", target_bir_lowering=False, debug=False)
    x_ap = nc.dram_tensor("x", [NS, C, S], F32, kind="ExternalInput").ap()
    cent_ap = nc.dram_tensor("centroids", [K, C], F32, kind="ExternalInput").ap()
    y_ap = nc.dram_tensor("y", [NS, K * C], F32, kind="ExternalOutput").ap()

    with tile.TileContext(nc) as tc:
        _body(tc, y_ap, x_ap, cent_ap)
    if compile:
        nc.compile()
    return nc


def _body(tc, y_ap, x_ap, cent_ap):
    nc = tc.nc
    from contextlib import ExitStack

    with ExitStack() as ctx:
        const = ctx.enter_context(tc.tile_pool(name="const", bufs=1))
        xin = ctx.enter_context(tc.tile_pool(name="xin", bufs=2))
        xsc = ctx.enter_context(tc.tile_pool(name="xsc", bufs=2))
        pers = ctx.enter_context(tc.tile_pool(name="pers", bufs=2))
        grp = ctx.enter_context(tc.tile_pool(name="grp", bufs=3))
        epi = ctx.enter_context(tc.tile_pool(name="epi", bufs=2))
        tps = ctx.enter_context(tc.tile_pool(name="tps", bufs=2, space="PSUM"))
        dps = ctx.enter_context(tc.tile_pool(name="dps", bufs=2, space="PSUM"))
        vps = ctx.enter_context(tc.tile_pool(name="vps", bufs=2, space="PSUM"))
        sps = ctx.enter_context(tc.tile_pool(name="sps", bufs=1, space="PSUM"))

        # ---------------- constants ----------------
        ident = const.tile([128, 128], F32)
        masks.make_identity(nc, ident[:])

        ones32 = const.tile([128, 1], F32)
        nc.gpsimd.memset(ones32[:], 1.0)
        ones_row = const.tile([1, 128], F32)
        nc.gpsimd.memset(ones_row[:], 1.0)

        cent_sb = const.tile([K, C], F32)
        nc.sync.dma_start(cent_sb[:], cent_ap)

        # centT [c, k] via PE transpose
        setup_ps = sps.tile([C, K], F32, tag="small_ps")
        nc.tensor.transpose(setup_ps[:], cent_sb[:], ident[0:K, 0:K])
        centT = const.tile([C, K], F32)
        nc.vector.tensor_copy(centT[:], setup_ps[:])

        # bias chain: bias_k = -a*||cent_k||ess; B = exp(bias - max bias)
        csq_scr = const.tile([K, C], BF16)
        cn2 = const.tile([K, 1], F32)
        nc.scalar.activation(csq_scr[:], cent_sb[:], AF.Square, accum_out=cn2[:])
        cnorm = const.tile([K, 1], F32)
        nc.scalar.activation(cnorm[:], cn2[:], AF.Sqrt)
        # bias_k = -ALPHA * ||cent_k||, materialized as [128, GT*K] broadcast
        nbias = const.tile([K, 1], F32)
        nc.vector.tensor_scalar_mul(nbias[:], cnorm[:], -ALPHA)
        biasr_ps = sps.tile([1, K], F32, tag="small_ps")
        nc.tensor.transpose(biasr_ps[:], nbias[:], ident[0:K, 0:K])
        biasr = const.tile([1, K], F32)
        nc.vector.tensor_copy(biasr[:], biasr_ps[:])
        bb_ps = sps.tile([128, K], F32, tag="small_ps")
        nc.tensor.matmul(bb_ps[:], ones_row[:], biasr[:], start=True, stop=True)
        bias4 = const.tile([128, GT * K], F32)
        for i in range(GT):
            nc.vector.tensor_copy(bias4[:, i * K : (i + 1) * K], bb_ps[:])

        # ---------------- main loop ----------------
        inv_4a2 = 1.0 / (4.0 * ALPHA * ALPHA)
        inv_2a = 1.0 / (2.0 * ALPHA)

        for n in range(NS):
            x_cs = xin.tile([C, S], F32)
            nc.sync.dma_start(x_cs[:], x_ap[n])

            x_sc = xsc.tile([128, S // ST, C], BF16)  # [s_in, s_out, c]
            ssq = pers.tile([128, S // ST], F32)
            # ---- pass 1: transpose + ssq ----
            for g in range(NG):
                t_ps = tps.tile([128, GT * ST], F32)
                sq_scr = grp.tile([128, GT * ST], BF16)
                for i in range(GT):
                    t = g * GT + i
                    nc.tensor.transpose(
                        t_ps[:, i * ST : (i + 1) * ST],
                        x_cs[:, t * ST : (t + 1) * ST],
                        ident[:],
                    )
                    # ssq for this tile (ACT square + accumulate)
                    nc.scalar.activation(
                        sq_scr[:, i * ST : (i + 1) * ST],
                        t_ps[:, i * ST : (i + 1) * ST],
                        AF.Square,
                        accum_out=ssq[:, t : t + 1],
                    )
                nc.vector.tensor_copy(
                    x_sc[:, g * GT : (g + 1) * GT, :],
                    t_ps[:].rearrange("p (i c) -> p i c", i=GT),
                )

            # r2 = 2a / sqrt(ssq):  sqrt(ssq/(4a^2)) then reciprocal
            sq_r = pers.tile([128, S // ST], F32)
            nc.scalar.activation(sq_r[:], ssq[:], AF.Sqrt, scale=inv_4a2)
            r2 = pers.tile([128, S // ST], F32)
            nc.vector.reciprocal(r2[:], sq_r[:])

            # ---- pass 2: logits, softmax, aggregation ----
            v_ps = vps.tile([K, C + 1], F32)
            for g in range(NG):
                d_ps = dps.tile([128, GT * K], F32)
                for i in range(GT):
                    t = g * GT + i
                    nc.tensor.matmul(
                        d_ps[:, i * K : (i + 1) * K],
                        x_cs[:, t * ST : (t + 1) * ST],
                        centT[:],
                        start=True,
                        stop=True,
                    )
                # l = r2*d + bias (joint logits, fp32)
                l_sb = grp.tile([128, GT * K], F32)
                for i in range(GT):
                    t = g * GT + i
                    nc.vector.scalar_tensor_tensor(
                        l_sb[:, i * K : (i + 1) * K],
                        d_ps[:, i * K : (i + 1) * K],
                        r2[:, t : t + 1],
                        bias4[:, i * K : (i + 1) * K],
                        op0=ALU.mult,
                        op1=ALU.add,
                    )
                # negated row max per tile
                nm = grp.tile([128, GT], F32)
                nc.vector.tensor_reduce(
                    nm[:],
                    l_sb[:].rearrange("p (i k) -> p i k", i=GT),
                    axis=mybir.AxisListType.X,
                    op=ALU.max,
                    negate=True,
                )
                e_sb = grp.tile([128, GT * K], BF16)
                u_sb = grp.tile([128, len(EXP_DVE) * K], F32)
                for j, i in enumerate(EXP_DVE):
                    nc.vector.tensor_scalar_add(
                        u_sb[:, j * K : (j + 1) * K],
                        l_sb[:, i * K : (i + 1) * K],
                        nm[:, i : i + 1],
                    )
                if EXP_DVE:
                    nc.scalar.activation(
                        e_sb[:, 0 : len(EXP_DVE) * K], u_sb[:], AF.Exp
                    )
                for i in EXP_FUSED:
                    nc.scalar.activation(
                        e_sb[:, i * K : (i + 1) * K],
                        l_sb[:, i * K : (i + 1) * K],
                        AF.Exp,
                        bias=nm[:, i : i + 1],
                    )

                ssum = grp.tile([128, GT], F32)
                nc.vector.tensor_reduce(
                    ssum[:],
                    e_sb[:].rearrange("p (i k) -> p i k", i=GT),
                    axis=mybir.AxisListType.X,
                    op=ALU.add,
                )
                inv = grp.tile([128, GT], F32)
                nc.vector.reciprocal(inv[:], ssum[:])
                rc = grp.tile([128, GT], F32)
                nc.vector.tensor_mul(rc[:], inv[:], r2[:, g * GT : (g + 1) * GT])

                # xn2[:, i, 0:128] = x_sc * rc/(2a) ; xn2[:, i, 128] = inv
                xn2 = grp.tile([128, GT, C + 1], BF16)
                for i in range(GT):
                    t = g * GT + i
                    nc.gpsimd.tensor_scalar(
                        xn2[:, i, 0:C],
                        x_sc[:, t, :],
                        rc[:, i : i + 1],
                        inv_2a,
                        op0=ALU.mult,
                        op1=ALU.mult,
                    )
                nc.vector.tensor_copy(
                    xn2[:, :, C : C + 1],
                    inv[:].rearrange("p (i o) -> p i o", o=1),
                )
                for i in range(GT):
                    nc.tensor.matmul(
                        v_ps[:],
                        e_sb[:, i * K : (i + 1) * K],
                        xn2[:, i, :],
                        start=(g == 0 and i == 0),
                        stop=(g == NG - 1 and i == GT - 1),
                    )

            # ---- epilogue ----
            asum_neg = epi.tile([K, 1], F32)
            nc.vector.tensor_scalar_mul(asum_neg[:], v_ps[:, C : C + 1], -1.0)
            vlad1 = epi.tile([K, C], F32)
            nc.vector.scalar_tensor_tensor(
                vlad1[:],
                cent_sb[:],
                asum_neg[:],
                v_ps[:, 0:C],
                op0=ALU.mult,
                op1=ALU.add,
            )
            sq1 = epi.tile([K, C], BF16)
            ss_k = epi.tile([K, 1], F32)
            nc.scalar.activation(sq1[:], vlad1[:], AF.Square, accum_out=ss_k[:])
            nrm = epi.tile([K, 1], F32)
            nc.scalar.activation(nrm[:], ss_k[:], AF.Sqrt)
            nrmc = epi.tile([K, 1], F32)
            nc.vector.tensor_scalar_max(nrmc[:], nrm[:], 1e-12)
            sck = epi.tile([K, 1], F32)
            nc.vector.reciprocal(sck[:], nrmc[:])
            vladn = epi.tile([K, C], F32)
            nc.vector.tensor_scalar_mul(vladn[:], vlad1[:], sck[:])
            sq2 = epi.tile([K, C], BF16)
            ss2 = epi.tile([K, 1], F32)
            nc.scalar.activation(sq2[:], vladn[:], AF.Square, accum_out=ss2[:])
            tot_ps = sps.tile([1, 1], F32, tag="small_ps")
            nc.tensor.matmul(tot_ps[:], ss2[:], ones32[0:K, :], start=True, stop=True)
            st = epi.tile([1, 1], F32)
            nc.scalar.activation(st[:], tot_ps[:], AF.Sqrt)
            stc = epi.tile([1, 1], F32)
            nc.vector.tensor_scalar_max(stc[:], st[:], 1e-12)
            sc2 = epi.tile([1, 1], F32)
            nc.vector.reciprocal(sc2[:], stc[:])
            sc2_ps = sps.tile([K, 1], F32, tag="small_ps")
            nc.tensor.matmul(
                sc2_ps[:], ones_row[:, 0:K], sc2[:], start=True, stop=True
            )
            sc2b = epi.tile([K, 1], F32)
            nc.vector.tensor_copy(sc2b[:], sc2_ps[:])
            y_t = epi.tile([K, C], F32)
            nc.vector.tensor_scalar_mul(y_t[:], vladn[:], sc2b[:])
            nc.sync.dma_start(
                y_ap[n : n + 1, :].rearrange("o (k c) -> (o k) c", k=K), y_t[:]
            )


_NC_CACHE = None


def _get_nc():
    global _NC_CACHE
    if _NC_CACHE is None:
        _NC_CACHE = build_nc()
    return _NC_CACHE


LAST_RESULTS = None


def kernel(x, centroids, trace=False, trace_kwargs=None):
    global LAST_RESULTS
    x = np.ascontiguousarray(np.asarray(x, dtype=np.float32))
    centroids = np.ascontiguousarray(np.asarray(centroids, dtype=np.float32))
    N = x.shape[0]
    xs = x.reshape(N, C, S)
    nc = _get_nc()
    per = N // N_CORES
    in_maps = [
        {"x": xs[i * per : (i + 1) * per], "centroids": centroids}
        for i in range(N_CORES)
    ]
    res = run_bass_kernel_spmd(
        nc,
        in_maps,
        core_ids=list(range(N_CORES)),
        trace=trace,
        **(trace_kwargs or {}),
    )
    LAST_RESULTS = res
    y = np.concatenate([r["y"] for r in res.results], axis=0)
    return y.astype(np.float32)


# revision 26
# speedup vs baseline: 1.0060x; 1.0060x over previous
"""NetVLAD Trainium2 Bass kernel (v3).

Per sample (C=128 channels, S=4096 spatial, K=64 clusters):
  xn = x / ||x||_c ;  l[s,k] = 2a*xn_s.c_k - a*||c_k|| ;  a = softmax_k(l)
  vlad[k,c] = sum_s a[s,k]*(xn[s,c] - c[k,c]); intra-norm rows; global l2.

Sharding: batch 32 -> 8 cores x 4 samples, centroids replicated, no
collectives; host concatenates. Inputs are re-encoded host-side as an
fp16 hi/lo pair (same total bytes as fp32) so the device GEMMs see
~fp32-accurate x without a separate cast pass.

Per-core dataflow, phased to keep ACT on one function table per phase:
  A (per sample): HWDGE loads xh/xlo [c,s]; DMA-transpose xh -> X_sc
     [s,8,c] fp16; ACT Square -> fp16 squares; DVE reduce -> ssq.
  r2 batch: ACT Sqrt + DVE reciprocal -> r2 = 2a/||x|| for all samples.
  B (per sample, 4 groups of 8 s-tiles): fp16 GEMM1 (xh*cT + xh*cT_lo +
     xlo*cT) -> d PSUM [128,8,64]; logits l = r2*d + bias via DVE TT
     (broadcast r2) + GPSIMD add (bias bcast tensor); DVE rowmax
     (negate); u = l - max via GPSIMD TT; one big ACT Exp -> e bf16;
     DVE ssum/inv/G; xn2 = X_sc*G (DVE TT bcast); bf16 GEMM2
     V[64,130] += e^T @ [xn2 | inv_hi | inv_lo]; V -> SBUF.
  C (per sample): vlad = V - asum*cent, intra-norm + global norm
     (ACT Sqrt, DVE max/recip), DMA out.
"""

import sys

import numpy as np

sys.path.insert(0, "/opt/trn_rl_repo")

import concourse.bacc as bacc  # noqa: E402
import concourse.bass as bass  # noqa: E402
import concourse.tile as tile  # noqa: E402
from concourse import mybir  # noqa: E402
from concourse.bass_utils import run_bass_kernel_spmd  # noqa: E402

F32 = mybir.dt.float32
F16 = mybir.dt.float16
BF16 = mybir.dt.bfloat16
AF = mybir.ActivationFunctionType
ALU = mybir.AluOpType

ALPHA = 100.0
N_CORES = 8
NS = 4  # samples per core
C = 128
S = 4096
K = 64
ST = 128  # s-tile (PSUM partition limit)
GT = 8  # tiles per group
NG = S // (ST * GT)  # 4 groups per sample
NT = S // ST  # 32 tiles per sample


def build_nc(compile=True):
    nc = bacc.Bacc("TRN2", target_bir_lowering=False, debug=False)
    xh_ap = nc.dram_tensor("xh", [NS, C, S], F16, kind="ExternalInput").ap()
    xlo_ap = nc.dram_tensor("xlo", [NS, C, S], F16, kind="ExternalInput").ap()
    cent_ap = nc.dram_tensor("centroids", [K, C], F32, kind="ExternalInput").ap()
    y_ap = nc.dram_tensor("y", [NS, K * C], F32, kind="ExternalOutput").ap()

    with tile.TileContext(nc) as tc:
        _body(tc, y_ap, xh_ap, xlo_ap, cent_ap)
    if compile:
        nc.compile()
    return nc


def _body(tc, y_ap, xh_ap, xlo_ap, cent_ap):
    nc = tc.nc
    from contextlib import ExitStack

    with ExitStack() as ctx:
        const = ctx.enter_context(tc.tile_pool(name="const", bufs=1))
        xin = ctx.enter_context(tc.tile_pool(name="xin", bufs=NS))
        xsc = ctx.enter_context(tc.tile_pool(name="xsc", bufs=NS))
        pers = ctx.enter_context(tc.tile_pool(name="pers", bufs=NS))
        grp = ctx.enter_context(tc.tile_pool(name="grp", bufs=3))
        epi = ctx.enter_context(tc.tile_pool(name="epi", bufs=2))
        dps = ctx.enter_context(tc.tile_pool(name="dps", bufs=2, space="PSUM"))
        vps = ctx.enter_context(tc.tile_pool(name="vps", bufs=2, space="PSUM"))
        sps = ctx.enter_context(tc.tile_pool(name="sps", bufs=1, space="PSUM"))

        # ---------------- constants ----------------
        ident = const.tile([K, K], F32)
        from concourse import masks

        masks.make_identity(nc, ident[:])
        ones_row = const.tile([1, 128], F32)
        nc.gpsimd.memset(ones_row[:], 1.0)
        ones_col = const.tile([K, 1], F32)
        nc.gpsimd.memset(ones_col[:], 1.0)

        cent_sb = const.tile([K, C], F32)
        nc.sync.dma_start(cent_sb[:], cent_ap)

        # centT fp16 hi/lo [c, k]
        setup_ps = sps.tile([C, K], F32, tag="small_ps")
        nc.tensor.transpose(setup_ps[:], cent_sb[:], ident[:])
        centTf = const.tile([C, K], F32)
        nc.vector.tensor_copy(centTf[:], setup_ps[:])
        centT = const.tile([C, K], F16)
        nc.vector.tensor_copy(centT[:], centTf[:])
        centT_lo = const.tile([C, K], F16)
        nc.vector.tensor_sub(centT_lo[:], centTf[:], centT[:])

        # bias_k = -ALPHA*||cent_k|| broadcast into [128, GT*K]
        csq_scr = const.tile([K, C], BF16)
        cn2 = const.tile([K, 1], F32)
        nc.scalar.activation(csq_scr[:], cent_sb[:], AF.Square, accum_out=cn2[:])
        cnorm = const.tile([K, 1], F32)
        nc.scalar.activation(cnorm[:], cn2[:], AF.Sqrt)
        nbias = const.tile([K, 1], F32)
        nc.vector.tensor_scalar_mul(nbias[:], cnorm[:], -ALPHA)
        biasr_ps = sps.tile([1, K], F32, tag="small_ps")
        nc.tensor.transpose(biasr_ps[:], nbias[:], ident[:])
        biasr = const.tile([1, K], F32)
        nc.vector.tensor_copy(biasr[:], biasr_ps[:])
        bb_ps = sps.tile([128, K], F32, tag="small_ps")
        nc.tensor.matmul(bb_ps[:], ones_row[:], biasr[:], start=True, stop=True)
        bias8 = const.tile([128, GT * K], F32)
        for i in range(GT):
            nc.vector.tensor_copy(bias8[:, i * K : (i + 1) * K], bb_ps[:])

        inv_4a2 = 1.0 / (4.0 * ALPHA * ALPHA)
        inv_2a = 1.0 / (2.0 * ALPHA)

        xh_t = []
        xlo_t = []
        xsc_t = []
        ssq_t = []
        r2_t = []
        vsb_t = []

        # ---------------- phase A: load, transpose, ssq ----------------
        for n in range(NS):
            xh = xin.tile([C, NG, GT * ST], F16, tag="xh")
            xlo = xin.tile([C, NG, GT * ST], F16, tag="xlo")
            xh_v = xh_ap[n].rearrange("c (g t) -> c g t", g=NG)
            xlo_v = xlo_ap[n].rearrange("c (g t) -> c g t", g=NG)
            x_sc = xsc.tile([128, NT, C], F16)  # [s_in, s_out, c]
            ssq_s = pers.tile([128, NT], F32, tag="ssq")
            for g in range(NG):
                nc.sync.dma_start(xh[:, g, :], xh_v[:, g, :])
                nc.sync.dma_start(xlo[:, g, :], xlo_v[:, g, :])
                nc.sync.dma_start_transpose(
                    x_sc[:, g * GT : (g + 1) * GT, :], xh[:, g, :]
                )
                xsq = grp.tile([128, GT, C], F16)
                nc.scalar.activation(
                    xsq[:], x_sc[:, g * GT : (g + 1) * GT, :], AF.Square
                )
                nc.vector.tensor_reduce(
                    ssq_s[:, g * GT : (g + 1) * GT],
                    xsq[:],
                    axis=mybir.AxisListType.X,
                    op=ALU.add,
                )
            xh_t.append(xh)
            xlo_t.append(xlo)
            xsc_t.append(x_sc)
            ssq_t.append(ssq_s)

        # ---------------- r2 batch (one Sqrt table visit) ----------------
        for n in range(NS):
            sqv = pers.tile([128, NT], F32, tag="sqv")
            nc.scalar.activation(sqv[:], ssq_t[n][:], AF.Sqrt, scale=inv_4a2)
            r2_s = pers.tile([128, NT], F32, tag="r2")
            nc.vector.reciprocal(r2_s[:], sqv[:])
            r2_t.append(r2_s)

        # ---------------- phase B: logits, softmax, aggregation ----------
        for n in range(NS):
            xh, xlo, x_sc, r2_s = xh_t[n], xlo_t[n], xsc_t[n], r2_t[n]
            v_ps = vps.tile([K, C + 2], F32)
            for g in range(NG):
                d_ps = dps.tile([128, GT, K], F32)
                for i in range(GT):
                    nc.tensor.matmul(
                        d_ps[:, i, :],
                        xh[:, g, bass.ts(i, ST)],
                        centT[:],
                        start=True,
                        stop=False,
                    )
                    nc.tensor.matmul(
                        d_ps[:, i, :],
                        xh[:, g, bass.ts(i, ST)],
                        centT_lo[:],
                        start=False,
                        stop=False,
                    )
                    nc.tensor.matmul(
                        d_ps[:, i, :],
                        xlo[:, g, bass.ts(i, ST)],
                        centT[:],
                        start=False,
                        stop=True,
                    )
                r2g = r2_s[:, g * GT : (g + 1) * GT]
                w_sb = grp.tile([128, GT, K], F32)
                nc.vector.tensor_tensor(
                    out=w_sb[:],
                    in0=d_ps[:],
                    in1=r2g.rearrange("p (i o) -> p i o", o=1).broadcast_to(
                        (128, GT, K)
                    ),
                    op=ALU.mult,
                )
                l_sb = grp.tile([128, GT, K], F32)
                nc.gpsimd.tensor_add(
                    l_sb[:], w_sb[:], bias8[:].rearrange("p (i k) -> p i k", i=GT)
                )
                nm = grp.tile([128, GT], F32)
                nc.vector.tensor_reduce(
                    nm[:],
                    l_sb[:],
                    axis=mybir.AxisListType.X,
                    op=ALU.max,
                    negate=True,
                )
                u2 = grp.tile([128, GT, K], F32)
                nc.gpsimd.tensor_tensor(
                    out=u2[:],
                    in0=l_sb[:],
                    in1=nm[:]
                    .rearrange("p (i o) -> p i o", o=1)
                    .broadcast_to((128, GT, K)),
                    op=ALU.add,
                )
                e_sb = grp.tile([128, GT, K], BF16)
                nc.scalar.activation(e_sb[:], u2[:], AF.Exp)

                ssum = grp.tile([128, GT], F32)
                nc.vector.tensor_reduce(
                    ssum[:], e_sb[:], axis=mybir.AxisListType.X, op=ALU.add
                )
                inv_i = grp.tile([128, GT], F32)
                nc.vector.reciprocal(inv_i[:], ssum[:])
                gsc = grp.tile([128, GT], BF16)
                nc.vector.scalar_tensor_tensor(
                    gsc[:],
                    r2g,
                    inv_2a,
                    inv_i[:],
                    op0=ALU.mult,
                    op1=ALU.mult,
                )
                # inv column hi/lo so asum is ~exact in the bf16 GEMM
                col = grp.tile([128, GT, 2], BF16)
                nc.vector.tensor_copy(col[:, :, 0], inv_i[:])
                nc.vector.tensor_sub(col[:, :, 1], inv_i[:], col[:, :, 0])

                xn2 = grp.tile([128, GT, C], BF16)
                nc.vector.tensor_tensor(
                    out=xn2[:],
                    in0=x_sc[:, g * GT : (g + 1) * GT, :],
                    in1=gsc[:]
                    .rearrange("p (i o) -> p i o", o=1)
                    .broadcast_to((128, GT, C)),
                    op=ALU.mult,
                )
                for i in range(GT):
                    first = g == 0 and i == 0
                    last = g == NG - 1 and i == GT - 1
                    nc.tensor.matmul(
                        v_ps[:, 0:C],
                        e_sb[:, i, :],
                        xn2[:, i, :],
                        start=first,
                        stop=False,
                    )
                    nc.tensor.matmul(
                        v_ps[:, C : C + 2],
                        e_sb[:, i, :],
                        col[:, i, :],
                        start=False,
                        stop=last,
                    )
            v_sb = pers.tile([K, C + 2], F32, tag="vsb")
            nc.vector.tensor_copy(v_sb[:], v_ps[:])
            vsb_t.append(v_sb)

        # ---------------- phase C: epilogues ----------------
        for n in range(NS):
            v_sb = vsb_t[n]
            asum_neg = epi.tile([K, 1], F32)
            nc.vector.tensor_reduce(
                asum_neg[:],
                v_sb[:, C : C + 2],
                axis=mybir.AxisListType.X,
                op=ALU.add,
                negate=True,
            )
            vlad1 = epi.tile([K, C], F32)
            nc.vector.scalar_tensor_tensor(
                vlad1[:],
                cent_sb[:],
                asum_neg[:],
                v_sb[:, 0:C],
                op0=ALU.mult,
                op1=ALU.add,
            )
            sq1 = epi.tile([K, C], BF16)
            ss_k = epi.tile([K, 1], F32)
            nc.scalar.activation(sq1[:], vlad1[:], AF.Square, accum_out=ss_k[:])
            nrm = epi.tile([K, 1], F32)
            nc.scalar.activation(nrm[:], ss_k[:], AF.Sqrt)
            nrmc = epi.tile([K, 1], F32)
            nc.vector.tensor_scalar_max(nrmc[:], nrm[:], 1e-12)
            sck = epi.tile([K, 1], F32)
            nc.vector.reciprocal(sck[:], nrmc[:])
            vladn = epi.tile([K, C], F32)
            nc.vector.tensor_scalar_mul(vladn[:], vlad1[:], sck[:])
            sq2 = epi.tile([K, C], BF16)
            ss2 = epi.tile([K, 1], F32)
            nc.scalar.activation(sq2[:], vladn[:], AF.Square, accum_out=ss2[:])
            tot_ps = sps.tile([1, 1], F32, tag="small_ps")
            nc.tensor.matmul(tot_ps[:], ss2[:], ones_col[:], start=True, stop=True)
            st = epi.tile([1, 1], F32)
            nc.scalar.activation(st[:], tot_ps[:], AF.Sqrt)
            stc = epi.tile([1, 1], F32)
            nc.vector.tensor_scalar_max(stc[:], st[:], 1e-12)
            sc2 = epi.tile([1, 1], F32)
            nc.vector.reciprocal(sc2[:], stc[:])
            sc2_ps = sps.tile([K, 1], F32, tag="small_ps")
            nc.tensor.matmul(
                sc2_ps[:], ones_row[:, 0:K], sc2[:], start=True, stop=True
            )
            sc2b = epi.tile([K, 1], F32)
            nc.vector.tensor_copy(sc2b[:], sc2_ps[:])
            y_t = epi.tile([K, C], F32)
            nc.vector.tensor_scalar_mul(y_t[:], vladn[:], sc2b[:])
            nc.sync.dma_start(
                y_ap[n : n + 1, :].rearrange("o (k c) -> (o k) c", k=K), y_t[:]
            )


_NC_CACHE = None


def _get_nc():
    global _NC_CACHE
    if _NC_CACHE is None:
        _NC_CACHE = build_nc()
    return _NC_CACHE


LAST_RESULTS = None


def kernel(x, centroids, trace=False, trace_kwargs=None):
    global LAST_RESULTS
    x = np.ascontiguousarray(np.asarray(x, dtype=np.float32))
    centroids = np.ascontiguousarray(np.asarray(centroids, dtype=np.float32))
    N = x.shape[0]
    xs = x.reshape(N, C, S)
    # lossless-ish fp16 hi/lo re-encoding of the input for DMA (same total
    # bytes as fp32); all NetVLAD arithmetic happens on device.
    xh = xs.astype(np.float16)
    xlo = (xs - xh.astype(np.float32)).astype(np.float16)
    nc = _get_nc()
    per = N // N_CORES
    in_maps = [
        {
            "xh": xh[i * per : (i + 1) * per],
            "xlo": xlo[i * per : (i + 1) * per],
            "centroids": centroids,
        }
        for i in range(N_CORES)
    ]
    res = run_bass_kernel_spmd(
        nc,
        in_maps,
        core_ids=list(range(N_CORES)),
        trace=trace,
        **(trace_kwargs or {}),
    )
    LAST_RESULTS = res
    y = np.concatenate([r["y"] for r in res.results], axis=0)
    return y.astype(np.float32)


# revision 27
# speedup vs baseline: 1.4882x; 1.4793x over previous
"""NetVLAD Trainium2 Bass kernel (v3).

Per sample (C=128 channels, S=4096 spatial, K=64 clusters):
  xn = x / ||x||_c ;  l[s,k] = 2a*xn_s.c_k - a*||c_k|| ;  a = softmax_k(l)
  vlad[k,c] = sum_s a[s,k]*(xn[s,c] - c[k,c]); intra-norm rows; global l2.

Sharding: batch 32 -> 8 cores x 4 samples, centroids replicated, no
collectives; host concatenates. Inputs are re-encoded host-side as an
fp16 hi/lo pair (same total bytes as fp32) so the device GEMMs see
~fp32-accurate x without a separate cast pass.

Per-core dataflow, phased to keep ACT on one function table per phase:
  A (per sample): HWDGE loads xh/xlo [c,s]; DMA-transpose xh -> X_sc
     [s,8,c] fp16; ACT Square -> fp16 squares; DVE reduce -> ssq.
  r2 batch: ACT Sqrt + DVE reciprocal -> r2 = 2a/||x|| for all samples.
  B (per sample, 4 groups of 8 s-tiles): fp16 GEMM1 (xh*cT + xh*cT_lo +
     xlo*cT) -> d PSUM [128,8,64]; logits l = r2*d + bias via DVE TT
     (broadcast r2) + GPSIMD add (bias bcast tensor); DVE rowmax
     (negate); u = l - max via GPSIMD TT; one big ACT Exp -> e bf16;
     DVE ssum/inv/G; xn2 = X_sc*G (DVE TT bcast); bf16 GEMM2
     V[64,130] += e^T @ [xn2 | inv_hi | inv_lo]; V -> SBUF.
  C (per sample): vlad = V - asum*cent, intra-norm + global norm
     (ACT Sqrt, DVE max/recip), DMA out.
"""

import sys

import numpy as np

sys.path.insert(0, "/opt/trn_rl_repo")

import concourse.bacc as bacc  # noqa: E402
import concourse.bass as bass  # noqa: E402
import concourse.tile as tile  # noqa: E402
from concourse import mybir  # noqa: E402
from concourse.bass_utils import run_bass_kernel_spmd  # noqa: E402

F32 = mybir.dt.float32
F16 = mybir.dt.float16
BF16 = mybir.dt.bfloat16
AF = mybir.ActivationFunctionType
ALU = mybir.AluOpType

ALPHA = 100.0
N_CORES = 8
NS = 4  # samples per core
C = 128
S = 4096
K = 64
ST = 128  # s-tile (PSUM partition limit)
GT = 8  # tiles per group
NG = S // (ST * GT)  # 4 groups per sample
NT = S // ST  # 32 tiles per sample


def build_nc(compile=True):
    nc = bacc.Bacc("TRN2", target_bir_lowering=False, debug=False)
    xh_ap = nc.dram_tensor("xh", [NS, C, S], F16, kind="ExternalInput").ap()
    xlo_ap = nc.dram_tensor("xlo", [NS, C, S], F16, kind="ExternalInput").ap()
    cent_ap = nc.dram_tensor("centroids", [K, C], F32, kind="ExternalInput").ap()
    y_ap = nc.dram_tensor("y", [NS, K * C], F32, kind="ExternalOutput").ap()

    with tile.TileContext(nc) as tc:
        _body(tc, y_ap, xh_ap, xlo_ap, cent_ap)
    if compile:
        nc.compile()
    return nc


def _body(tc, y_ap, xh_ap, xlo_ap, cent_ap):
    nc = tc.nc
    from contextlib import ExitStack

    with ExitStack() as ctx:
        const = ctx.enter_context(tc.tile_pool(name="const", bufs=1))
        xin = ctx.enter_context(tc.tile_pool(name="xin", bufs=NS))
        xsc = ctx.enter_context(tc.tile_pool(name="xsc", bufs=NS))
        pers = ctx.enter_context(tc.tile_pool(name="pers", bufs=NS))
        grp = ctx.enter_context(tc.tile_pool(name="grp", bufs=3))
        epi = ctx.enter_context(tc.tile_pool(name="epi", bufs=2))
        dps = ctx.enter_context(tc.tile_pool(name="dps", bufs=2, space="PSUM"))
        vps = ctx.enter_context(tc.tile_pool(name="vps", bufs=2, space="PSUM"))
        sps = ctx.enter_context(tc.tile_pool(name="sps", bufs=1, space="PSUM"))

        # ---------------- constants ----------------
        ident = const.tile([K, K], F32)
        from concourse import masks

        masks.make_identity(nc, ident[:])
        ones_row = const.tile([1, 128], F32)
        nc.gpsimd.memset(ones_row[:], 1.0)
        ones_col = const.tile([K, 1], F32)
        nc.gpsimd.memset(ones_col[:], 1.0)

        cent_sb = const.tile([K, C], F32)
        nc.sync.dma_start(cent_sb[:], cent_ap)

        # centT fp16 hi/lo [c, k]
        setup_ps = sps.tile([C, K], F32, tag="small_ps")
        nc.tensor.transpose(setup_ps[:], cent_sb[:], ident[:])
        centTf = const.tile([C, K], F32)
        nc.vector.tensor_copy(centTf[:], setup_ps[:])
        centT = const.tile([C, K], F16)
        nc.vector.tensor_copy(centT[:], centTf[:])
        centT_lo = const.tile([C, K], F16)
        nc.vector.tensor_sub(centT_lo[:], centTf[:], centT[:])

        # bias_k = -ALPHA*||cent_k|| broadcast into [128, GT*K]
        csq_scr = const.tile([K, C], BF16)
        cn2 = const.tile([K, 1], F32)
        nc.scalar.activation(csq_scr[:], cent_sb[:], AF.Square, accum_out=cn2[:])
        cnorm = const.tile([K, 1], F32)
        nc.scalar.activation(cnorm[:], cn2[:], AF.Sqrt)
        nbias = const.tile([K, 1], F32)
        nc.vector.tensor_scalar_mul(nbias[:], cnorm[:], -ALPHA)
        biasr_ps = sps.tile([1, K], F32, tag="small_ps")
        nc.tensor.transpose(biasr_ps[:], nbias[:], ident[:])
        biasr = const.tile([1, K], F32)
        nc.vector.tensor_copy(biasr[:], biasr_ps[:])
        bb_ps = sps.tile([128, K], F32, tag="small_ps")
        nc.tensor.matmul(bb_ps[:], ones_row[:], biasr[:], start=True, stop=True)
        bias8 = const.tile([128, GT * K], F32)
        for i in range(GT):
            nc.vector.tensor_copy(bias8[:, i * K : (i + 1) * K], bb_ps[:])

        inv_4a2 = 1.0 / (4.0 * ALPHA * ALPHA)
        inv_2a = 1.0 / (2.0 * ALPHA)

        xh_t = []
        xlo_t = []
        xsc_t = []
        ssq_t = []
        r2_t = []
        vsb_t = []

        # ---------------- phase A: load, transpose, ssq ----------------
        # All plain DMAs for a sample pair are issued before any xbar
        # transpose (the hardware serializes on DMA xbar-mode switches),
        # and phase B of earlier pairs overlaps phase A DMA of later ones.
        def phase_a(n):
            xh = xin.tile([C, NG, GT * ST], F16, tag="xh")
            xlo = xin.tile([C, NG, GT * ST], F16, tag="xlo")
            xh_v = xh_ap[n].rearrange("c (g t) -> c g t", g=NG)
            xlo_v = xlo_ap[n].rearrange("c (g t) -> c g t", g=NG)
            for g in range(NG):
                nc.sync.dma_start(xh[:, g, :], xh_v[:, g, :])
                nc.sync.dma_start(xlo[:, g, :], xlo_v[:, g, :])
            xh_t.append(xh)
            xlo_t.append(xlo)

        def phase_a2(n):
            xh = xh_t[n]
            x_sc = xsc.tile([128, NT, C], F16)  # [s_in, s_out, c]
            for g in range(NG):
                nc.sync.dma_start_transpose(
                    x_sc[:, g * GT : (g + 1) * GT, :], xh[:, g, :]
                )
            xsc_t.append(x_sc)

        def phase_ssq(n):
            x_sc = xsc_t[n]
            ssq_s = pers.tile([128, NT], F32, tag="ssq")
            for g in range(NG):
                xsq = grp.tile([128, GT, C], F16)
                nc.scalar.activation(
                    xsq[:], x_sc[:, g * GT : (g + 1) * GT, :], AF.Square
                )
                nc.vector.tensor_reduce(
                    ssq_s[:, g * GT : (g + 1) * GT],
                    xsq[:],
                    axis=mybir.AxisListType.X,
                    op=ALU.add,
                )
            sqv = pers.tile([128, NT], F32, tag="sqv")
            nc.scalar.activation(sqv[:], ssq_s[:], AF.Sqrt, scale=inv_4a2)
            r2_s = pers.tile([128, NT], F32, tag="r2")
            nc.vector.reciprocal(r2_s[:], sqv[:])
            r2_t.append(r2_s)

        # ---------------- phase B: logits, softmax, aggregation ----------
        def phase_b(n):
            xh, xlo, x_sc, r2_s = xh_t[n], xlo_t[n], xsc_t[n], r2_t[n]
            v_ps = vps.tile([K, C + 2], F32)
            for g in range(NG):
                d_ps = dps.tile([128, GT, K], F32)
                for i in range(GT):
                    nc.tensor.matmul(
                        d_ps[:, i, :],
                        xh[:, g, bass.ts(i, ST)],
                        centT[:],
                        start=True,
                        stop=False,
                    )
                    nc.tensor.matmul(
                        d_ps[:, i, :],
                        xh[:, g, bass.ts(i, ST)],
                        centT_lo[:],
                        start=False,
                        stop=False,
                    )
                    nc.tensor.matmul(
                        d_ps[:, i, :],
                        xlo[:, g, bass.ts(i, ST)],
                        centT[:],
                        start=False,
                        stop=True,
                    )
                r2g = r2_s[:, g * GT : (g + 1) * GT]
                w_sb = grp.tile([128, GT, K], F32)
                nc.vector.tensor_tensor(
                    out=w_sb[:],
                    in0=d_ps[:],
                    in1=r2g.rearrange("p (i o) -> p i o", o=1).broadcast_to(
                        (128, GT, K)
                    ),
                    op=ALU.mult,
                )
                l_sb = grp.tile([128, GT, K], F32)
                nc.gpsimd.tensor_add(
                    l_sb[:], w_sb[:], bias8[:].rearrange("p (i k) -> p i k", i=GT)
                )
                nm = grp.tile([128, GT], F32)
                nc.vector.tensor_reduce(
                    nm[:],
                    l_sb[:],
                    axis=mybir.AxisListType.X,
                    op=ALU.max,
                    negate=True,
                )
                u2 = grp.tile([128, GT, K], F32)
                nc.gpsimd.tensor_tensor(
                    out=u2[:],
                    in0=l_sb[:],
                    in1=nm[:]
                    .rearrange("p (i o) -> p i o", o=1)
                    .broadcast_to((128, GT, K)),
                    op=ALU.add,
                )
                e_sb = grp.tile([128, GT, K], BF16)
                nc.scalar.activation(e_sb[:], u2[:], AF.Exp)

                ssum = grp.tile([128, GT], F32)
                nc.vector.tensor_reduce(
                    ssum[:], e_sb[:], axis=mybir.AxisListType.X, op=ALU.add
                )
                inv_i = grp.tile([128, GT], F32)
                nc.vector.reciprocal(inv_i[:], ssum[:])
                gsc = grp.tile([128, GT], BF16)
                nc.vector.scalar_tensor_tensor(
                    gsc[:],
                    r2g,
                    inv_2a,
                    inv_i[:],
                    op0=ALU.mult,
                    op1=ALU.mult,
                )
                # inv column hi/lo so asum is ~exact in the bf16 GEMM
                col = grp.tile([128, GT, 2], BF16)
                nc.vector.tensor_copy(col[:, :, 0], inv_i[:])
                nc.vector.tensor_sub(col[:, :, 1], inv_i[:], col[:, :, 0])

                xn2 = grp.tile([128, GT, C], BF16)
                nc.vector.tensor_tensor(
                    out=xn2[:],
                    in0=x_sc[:, g * GT : (g + 1) * GT, :],
                    in1=gsc[:]
                    .rearrange("p (i o) -> p i o", o=1)
                    .broadcast_to((128, GT, C)),
                    op=ALU.mult,
                )
                for i in range(GT):
                    first = g == 0 and i == 0
                    last = g == NG - 1 and i == GT - 1
                    nc.tensor.matmul(
                        v_ps[:, 0:C],
                        e_sb[:, i, :],
                        xn2[:, i, :],
                        start=first,
                        stop=False,
                    )
                    nc.tensor.matmul(
                        v_ps[:, C : C + 2],
                        e_sb[:, i, :],
                        col[:, i, :],
                        start=False,
                        stop=last,
                    )
            v_sb = pers.tile([K, C + 2], F32, tag="vsb")
            nc.vector.tensor_copy(v_sb[:], v_ps[:])
            vsb_t.append(v_sb)

        # software pipeline: A(0,1) | A(2,3) overlapping B(0,1) | B(2,3)
        phase_a(0)
        phase_a(1)
        phase_a2(0)
        phase_a2(1)
        phase_ssq(0)
        phase_ssq(1)
        phase_a(2)
        phase_a(3)
        phase_a2(2)
        phase_a2(3)
        phase_b(0)
        phase_ssq(2)
        phase_ssq(3)
        phase_b(1)
        phase_b(2)
        phase_b(3)

        # ---------------- phase C: epilogues ----------------
        for n in range(NS):
            v_sb = vsb_t[n]
            asum_neg = epi.tile([K, 1], F32)
            nc.vector.tensor_reduce(
                asum_neg[:],
                v_sb[:, C : C + 2],
                axis=mybir.AxisListType.X,
                op=ALU.add,
                negate=True,
            )
            vlad1 = epi.tile([K, C], F32)
            nc.vector.scalar_tensor_tensor(
                vlad1[:],
                cent_sb[:],
                asum_neg[:],
                v_sb[:, 0:C],
                op0=ALU.mult,
                op1=ALU.add,
            )
            sq1 = epi.tile([K, C], BF16)
            ss_k = epi.tile([K, 1], F32)
            nc.scalar.activation(sq1[:], vlad1[:], AF.Square, accum_out=ss_k[:])
            nrm = epi.tile([K, 1], F32)
            nc.scalar.activation(nrm[:], ss_k[:], AF.Sqrt)
            nrmc = epi.tile([K, 1], F32)
            nc.vector.tensor_scalar_max(nrmc[:], nrm[:], 1e-12)
            sck = epi.tile([K, 1], F32)
            nc.vector.reciprocal(sck[:], nrmc[:])
            vladn = epi.tile([K, C], F32)
            nc.vector.tensor_scalar_mul(vladn[:], vlad1[:], sck[:])
            sq2 = epi.tile([K, C], BF16)
            ss2 = epi.tile([K, 1], F32)
            nc.scalar.activation(sq2[:], vladn[:], AF.Square, accum_out=ss2[:])
            tot_ps = sps.tile([1, 1], F32, tag="small_ps")
            nc.tensor.matmul(tot_ps[:], ss2[:], ones_col[:], start=True, stop=True)
            st = epi.tile([1, 1], F32)
            nc.scalar.activation(st[:], tot_ps[:], AF.Sqrt)
            stc = epi.tile([1, 1], F32)
            nc.vector.tensor_scalar_max(stc[:], st[:], 1e-12)
            sc2 = epi.tile([1, 1], F32)
            nc.vector.reciprocal(sc2[:], stc[:])
            sc2_ps = sps.tile([K, 1], F32, tag="small_ps")
            nc.tensor.matmul(
                sc2_ps[:], ones_row[:, 0:K], sc2[:], start=True, stop=True
            )
            sc2b = epi.tile([K, 1], F32)
            nc.vector.tensor_copy(sc2b[:], sc2_ps[:])
            y_t = epi.tile([K, C], F32)
            nc.vector.tensor_scalar_mul(y_t[:], vladn[:], sc2b[:])
            nc.sync.dma_start(
                y_ap[n : n + 1, :].rearrange("o (k c) -> (o k) c", k=K), y_t[:]
            )


_NC_CACHE = None


def _get_nc():
    global _NC_CACHE
    if _NC_CACHE is None:
        _NC_CACHE = build_nc()
    return _NC_CACHE


LAST_RESULTS = None


def kernel(x, centroids, trace=False, trace_kwargs=None):
    global LAST_RESULTS
    x = np.ascontiguousarray(np.asarray(x, dtype=np.float32))
    centroids = np.ascontiguousarray(np.asarray(centroids, dtype=np.float32))
    N = x.shape[0]
    xs = x.reshape(N, C, S)
    # lossless-ish fp16 hi/lo re-encoding of the input for DMA (same total
    # bytes as fp32); all NetVLAD arithmetic happens on device.
    xh = xs.astype(np.float16)
    xlo = (xs - xh.astype(np.float32)).astype(np.float16)
    nc = _get_nc()
    per = N // N_CORES
    in_maps = [
        {
            "xh": xh[i * per : (i + 1) * per],
            "xlo": xlo[i * per : (i + 1) * per],
            "centroids": centroids,
        }
        for i in range(N_CORES)
    ]
    res = run_bass_kernel_spmd(
        nc,
        in_maps,
        core_ids=list(range(N_CORES)),
        trace=trace,
        **(trace_kwargs or {}),
    )
    LAST_RESULTS = res
    y = np.concatenate([r["y"] for r in res.results], axis=0)
    return y.astype(np.float32)


# revision 28
# speedup vs baseline: 1.5108x; 1.0152x over previous
"""NetVLAD Trainium2 Bass kernel (v3).

Per sample (C=128 channels, S=4096 spatial, K=64 clusters):
  xn = x / ||x||_c ;  l[s,k] = 2a*xn_s.c_k - a*||c_k|| ;  a = softmax_k(l)
  vlad[k,c] = sum_s a[s,k]*(xn[s,c] - c[k,c]); intra-norm rows; global l2.

Sharding: batch 32 -> 8 cores x 4 samples, centroids replicated, no
collectives; host concatenates. Inputs are re-encoded host-side as an
fp16 hi/lo pair (same total bytes as fp32) so the device GEMMs see
~fp32-accurate x without a separate cast pass.

Per-core dataflow, phased to keep ACT on one function table per phase:
  A (per sample): HWDGE loads xh/xlo [c,s]; DMA-transpose xh -> X_sc
     [s,8,c] fp16; ACT Square -> fp16 squares; DVE reduce -> ssq.
  r2 batch: ACT Sqrt + DVE reciprocal -> r2 = 2a/||x|| for all samples.
  B (per sample, 4 groups of 8 s-tiles): fp16 GEMM1 (xh*cT + xh*cT_lo +
     xlo*cT) -> d PSUM [128,8,64]; logits l = r2*d + bias via DVE TT
     (broadcast r2) + GPSIMD add (bias bcast tensor); DVE rowmax
     (negate); u = l - max via GPSIMD TT; one big ACT Exp -> e bf16;
     DVE ssum/inv/G; xn2 = X_sc*G (DVE TT bcast); bf16 GEMM2
     V[64,130] += e^T @ [xn2 | inv_hi | inv_lo]; V -> SBUF.
  C (per sample): vlad = V - asum*cent, intra-norm + global norm
     (ACT Sqrt, DVE max/recip), DMA out.
"""

import sys

import numpy as np

sys.path.insert(0, "/opt/trn_rl_repo")

import concourse.bacc as bacc  # noqa: E402
import concourse.bass as bass  # noqa: E402
import concourse.tile as tile  # noqa: E402
from concourse import mybir  # noqa: E402
from concourse.bass_utils import run_bass_kernel_spmd  # noqa: E402

F32 = mybir.dt.float32
F16 = mybir.dt.float16
BF16 = mybir.dt.bfloat16
AF = mybir.ActivationFunctionType
ALU = mybir.AluOpType

ALPHA = 100.0
N_CORES = 8
NS = 4  # samples per core
C = 128
S = 4096
K = 64
ST = 128  # s-tile (PSUM partition limit)
GT = 8  # tiles per group
NG = S // (ST * GT)  # 4 groups per sample
NT = S // ST  # 32 tiles per sample


def build_nc(compile=True):
    nc = bacc.Bacc("TRN2", target_bir_lowering=False, debug=False)
    xh_ap = nc.dram_tensor("xh", [NS, C, S], F16, kind="ExternalInput").ap()
    xlo_ap = nc.dram_tensor("xlo", [NS, C, S], F16, kind="ExternalInput").ap()
    cent_ap = nc.dram_tensor("centroids", [K, C], F32, kind="ExternalInput").ap()
    y_ap = nc.dram_tensor("y", [NS, K * C], F32, kind="ExternalOutput").ap()

    with tile.TileContext(nc) as tc:
        _body(tc, y_ap, xh_ap, xlo_ap, cent_ap)
    if compile:
        nc.compile()
    return nc


def _body(tc, y_ap, xh_ap, xlo_ap, cent_ap):
    nc = tc.nc
    from contextlib import ExitStack

    with ExitStack() as ctx:
        const = ctx.enter_context(tc.tile_pool(name="const", bufs=1))
        xin = ctx.enter_context(tc.tile_pool(name="xin", bufs=NS))
        xsc = ctx.enter_context(tc.tile_pool(name="xsc", bufs=NS))
        pers = ctx.enter_context(tc.tile_pool(name="pers", bufs=NS))
        grp = ctx.enter_context(tc.tile_pool(name="grp", bufs=3))
        epi = ctx.enter_context(tc.tile_pool(name="epi", bufs=2))
        dps = ctx.enter_context(tc.tile_pool(name="dps", bufs=2, space="PSUM"))
        vps = ctx.enter_context(tc.tile_pool(name="vps", bufs=2, space="PSUM"))
        sps = ctx.enter_context(tc.tile_pool(name="sps", bufs=1, space="PSUM"))

        # ---------------- constants ----------------
        ident = const.tile([K, K], F32)
        from concourse import masks

        masks.make_identity(nc, ident[:])
        ones_row = const.tile([1, 128], F32)
        nc.gpsimd.memset(ones_row[:], 1.0)
        ones_col = const.tile([K, 1], F32)
        nc.gpsimd.memset(ones_col[:], 1.0)

        cent_sb = const.tile([K, C], F32)
        nc.sync.dma_start(cent_sb[:], cent_ap)

        # centT fp16 hi/lo [c, k]
        setup_ps = sps.tile([C, K], F32, tag="small_ps")
        nc.tensor.transpose(setup_ps[:], cent_sb[:], ident[:])
        centTf = const.tile([C, K], F32)
        nc.vector.tensor_copy(centTf[:], setup_ps[:])
        centT = const.tile([C, K], F16)
        nc.vector.tensor_copy(centT[:], centTf[:])
        centT_lo = const.tile([C, K], F16)
        nc.vector.tensor_sub(centT_lo[:], centTf[:], centT[:])

        # bias_k = -ALPHA*||cent_k|| broadcast into [128, GT*K]
        csq_scr = const.tile([K, C], BF16)
        cn2 = const.tile([K, 1], F32)
        nc.scalar.activation(csq_scr[:], cent_sb[:], AF.Square, accum_out=cn2[:])
        cnorm = const.tile([K, 1], F32)
        nc.scalar.activation(cnorm[:], cn2[:], AF.Sqrt)
        nbias = const.tile([K, 1], F32)
        nc.vector.tensor_scalar_mul(nbias[:], cnorm[:], -ALPHA)
        biasr_ps = sps.tile([1, K], F32, tag="small_ps")
        nc.tensor.transpose(biasr_ps[:], nbias[:], ident[:])
        biasr = const.tile([1, K], F32)
        nc.vector.tensor_copy(biasr[:], biasr_ps[:])
        bb_ps = sps.tile([128, K], F32, tag="small_ps")
        nc.tensor.matmul(bb_ps[:], ones_row[:], biasr[:], start=True, stop=True)
        bias8 = const.tile([128, GT * K], F32)
        for i in range(GT):
            nc.vector.tensor_copy(bias8[:, i * K : (i + 1) * K], bb_ps[:])

        inv_4a2 = 1.0 / (4.0 * ALPHA * ALPHA)
        inv_2a = 1.0 / (2.0 * ALPHA)

        xh_t = []
        xlo_t = []
        xsc_t = []
        ssq_t = []
        r2_t = []
        vsb_t = []

        # ---------------- phase A: load, transpose, ssq ----------------
        # All plain DMAs for a sample pair are issued before any xbar
        # transpose (the hardware serializes on DMA xbar-mode switches),
        # and phase B of earlier pairs overlaps phase A DMA of later ones.
        def phase_a(n):
            xh = xin.tile([C, NG, GT * ST], F16, tag="xh")
            xlo = xin.tile([C, NG, GT * ST], F16, tag="xlo")
            xh_v = xh_ap[n].rearrange("c (g t) -> c g t", g=NG)
            xlo_v = xlo_ap[n].rearrange("c (g t) -> c g t", g=NG)
            for g in range(NG):
                nc.sync.dma_start(xh[:, g, :], xh_v[:, g, :])
                nc.sync.dma_start(xlo[:, g, :], xlo_v[:, g, :])
            xh_t.append(xh)
            xlo_t.append(xlo)

        def phase_a2(n):
            xh = xh_t[n]
            x_sc = xsc.tile([128, NT, C], F16)  # [s_in, s_out, c]
            for g in range(NG):
                nc.sync.dma_start_transpose(
                    x_sc[:, g * GT : (g + 1) * GT, :], xh[:, g, :]
                )
            xsc_t.append(x_sc)

        def phase_ssq(n):
            x_sc = xsc_t[n]
            ssq_s = pers.tile([128, NT], F32, tag="ssq")
            for g in range(NG):
                xsq = grp.tile([128, GT, C], F16)
                nc.scalar.activation(
                    xsq[:], x_sc[:, g * GT : (g + 1) * GT, :], AF.Square
                )
                nc.vector.tensor_reduce(
                    ssq_s[:, g * GT : (g + 1) * GT],
                    xsq[:],
                    axis=mybir.AxisListType.X,
                    op=ALU.add,
                )
            sqv = pers.tile([128, NT], F32, tag="sqv")
            nc.scalar.activation(sqv[:], ssq_s[:], AF.Sqrt, scale=inv_4a2)
            r2_s = pers.tile([128, NT], F32, tag="r2")
            nc.vector.reciprocal(r2_s[:], sqv[:])
            r2_t.append(r2_s)

        # ---------------- phase B: logits, softmax, aggregation ----------
        def phase_b(n):
            xh, xlo, x_sc, r2_s = xh_t[n], xlo_t[n], xsc_t[n], r2_t[n]
            v_ps = vps.tile([K, C + 2], F32)
            for g in range(NG):
                d_ps = dps.tile([128, GT, K], F32)
                for i in range(GT):
                    nc.tensor.matmul(
                        d_ps[:, i, :],
                        xh[:, g, bass.ts(i, ST)],
                        centT[:],
                        start=True,
                        stop=False,
                    )
                    nc.tensor.matmul(
                        d_ps[:, i, :],
                        xh[:, g, bass.ts(i, ST)],
                        centT_lo[:],
                        start=False,
                        stop=False,
                    )
                    nc.tensor.matmul(
                        d_ps[:, i, :],
                        xlo[:, g, bass.ts(i, ST)],
                        centT[:],
                        start=False,
                        stop=True,
                    )
                r2g = r2_s[:, g * GT : (g + 1) * GT]
                w_sb = grp.tile([128, GT, K], F32)
                nc.vector.tensor_tensor(
                    out=w_sb[:],
                    in0=d_ps[:],
                    in1=r2g.rearrange("p (i o) -> p i o", o=1).broadcast_to(
                        (128, GT, K)
                    ),
                    op=ALU.mult,
                )
                l_sb = grp.tile([128, GT, K], F32)
                nc.gpsimd.tensor_add(
                    l_sb[:], w_sb[:], bias8[:].rearrange("p (i k) -> p i k", i=GT)
                )
                nm = grp.tile([128, GT], F32)
                nc.vector.tensor_reduce(
                    nm[:],
                    l_sb[:],
                    axis=mybir.AxisListType.X,
                    op=ALU.max,
                    negate=True,
                )
                u2 = grp.tile([128, GT, K], F32)
                nc.gpsimd.tensor_tensor(
                    out=u2[:],
                    in0=l_sb[:],
                    in1=nm[:]
                    .rearrange("p (i o) -> p i o", o=1)
                    .broadcast_to((128, GT, K)),
                    op=ALU.add,
                )
                e_sb = grp.tile([128, GT, K], BF16)
                nc.scalar.activation(e_sb[:], u2[:], AF.Exp)

                ssum = grp.tile([128, GT], F32)
                nc.vector.tensor_reduce(
                    ssum[:], e_sb[:], axis=mybir.AxisListType.X, op=ALU.add
                )
                inv_i = grp.tile([128, GT], F32)
                nc.vector.reciprocal(inv_i[:], ssum[:])
                gsc = grp.tile([128, GT], BF16)
                nc.vector.scalar_tensor_tensor(
                    gsc[:],
                    r2g,
                    inv_2a,
                    inv_i[:],
                    op0=ALU.mult,
                    op1=ALU.mult,
                )
                # rhs = [xn2 | inv_hi | inv_lo]: one GEMM2 matmul per tile
                xn2 = grp.tile([128, GT, C + 2], BF16)
                nc.vector.tensor_copy(xn2[:, :, C], inv_i[:])
                nc.vector.tensor_sub(xn2[:, :, C + 1], inv_i[:], xn2[:, :, C])
                nc.vector.tensor_tensor(
                    out=xn2[:, :, 0:C],
                    in0=x_sc[:, g * GT : (g + 1) * GT, :],
                    in1=gsc[:]
                    .rearrange("p (i o) -> p i o", o=1)
                    .broadcast_to((128, GT, C)),
                    op=ALU.mult,
                )
                for i in range(GT):
                    first = g == 0 and i == 0
                    last = g == NG - 1 and i == GT - 1
                    nc.tensor.matmul(
                        v_ps[:],
                        e_sb[:, i, :],
                        xn2[:, i, :],
                        start=first,
                        stop=last,
                    )
            v_sb = pers.tile([K, C + 2], F32, tag="vsb")
            nc.vector.tensor_copy(v_sb[:], v_ps[:])
            vsb_t.append(v_sb)

        # software pipeline: A(0,1) | A(2,3) overlapping B(0,1) | B(2,3)
        phase_a(0)
        phase_a(1)
        phase_a2(0)
        phase_a2(1)
        phase_ssq(0)
        phase_ssq(1)
        phase_a(2)
        phase_a(3)
        phase_a2(2)
        phase_a2(3)
        phase_b(0)
        phase_ssq(2)
        phase_ssq(3)
        phase_b(1)
        phase_b(2)
        phase_b(3)

        # ---------------- phase C: epilogues ----------------
        for n in range(NS):
            v_sb = vsb_t[n]
            asum_neg = epi.tile([K, 1], F32)
            nc.vector.tensor_reduce(
                asum_neg[:],
                v_sb[:, C : C + 2],
                axis=mybir.AxisListType.X,
                op=ALU.add,
                negate=True,
            )
            vlad1 = epi.tile([K, C], F32)
            nc.vector.scalar_tensor_tensor(
                vlad1[:],
                cent_sb[:],
                asum_neg[:],
                v_sb[:, 0:C],
                op0=ALU.mult,
                op1=ALU.add,
            )
            sq1 = epi.tile([K, C], BF16)
            ss_k = epi.tile([K, 1], F32)
            nc.scalar.activation(sq1[:], vlad1[:], AF.Square, accum_out=ss_k[:])
            nrm = epi.tile([K, 1], F32)
            nc.scalar.activation(nrm[:], ss_k[:], AF.Sqrt)
            nrmc = epi.tile([K, 1], F32)
            nc.vector.tensor_scalar_max(nrmc[:], nrm[:], 1e-12)
            sck = epi.tile([K, 1], F32)
            nc.vector.reciprocal(sck[:], nrmc[:])
            vladn = epi.tile([K, C], F32)
            nc.vector.tensor_scalar_mul(vladn[:], vlad1[:], sck[:])
            sq2 = epi.tile([K, C], BF16)
            ss2 = epi.tile([K, 1], F32)
            nc.scalar.activation(sq2[:], vladn[:], AF.Square, accum_out=ss2[:])
            tot_ps = sps.tile([1, 1], F32, tag="small_ps")
            nc.tensor.matmul(tot_ps[:], ss2[:], ones_col[:], start=True, stop=True)
            st = epi.tile([1, 1], F32)
            nc.scalar.activation(st[:], tot_ps[:], AF.Sqrt)
            stc = epi.tile([1, 1], F32)
            nc.vector.tensor_scalar_max(stc[:], st[:], 1e-12)
            sc2 = epi.tile([1, 1], F32)
            nc.vector.reciprocal(sc2[:], stc[:])
            sc2_ps = sps.tile([K, 1], F32, tag="small_ps")
            nc.tensor.matmul(
                sc2_ps[:], ones_row[:, 0:K], sc2[:], start=True, stop=True
            )
            sc2b = epi.tile([K, 1], F32)
            nc.vector.tensor_copy(sc2b[:], sc2_ps[:])
            y_t = epi.tile([K, C], F32)
            nc.vector.tensor_scalar_mul(y_t[:], vladn[:], sc2b[:])
            nc.sync.dma_start(
                y_ap[n : n + 1, :].rearrange("o (k c) -> (o k) c", k=K), y_t[:]
            )


_NC_CACHE = None


def _get_nc():
    global _NC_CACHE
    if _NC_CACHE is None:
        _NC_CACHE = build_nc()
    return _NC_CACHE


LAST_RESULTS = None


def kernel(x, centroids, trace=False, trace_kwargs=None):
    global LAST_RESULTS
    x = np.ascontiguousarray(np.asarray(x, dtype=np.float32))
    centroids = np.ascontiguousarray(np.asarray(centroids, dtype=np.float32))
    N = x.shape[0]
    xs = x.reshape(N, C, S)
    # lossless-ish fp16 hi/lo re-encoding of the input for DMA (same total
    # bytes as fp32); all NetVLAD arithmetic happens on device.
    xh = xs.astype(np.float16)
    xlo = (xs - xh.astype(np.float32)).astype(np.float16)
    nc = _get_nc()
    per = N // N_CORES
    in_maps = [
        {
            "xh": xh[i * per : (i + 1) * per],
            "xlo": xlo[i * per : (i + 1) * per],
            "centroids": centroids,
        }
        for i in range(N_CORES)
    ]
    res = run_bass_kernel_spmd(
        nc,
        in_maps,
        core_ids=list(range(N_CORES)),
        trace=trace,
        **(trace_kwargs or {}),
    )
    LAST_RESULTS = res
    y = np.concatenate([r["y"] for r in res.results], axis=0)
    return y.astype(np.float32)
